# revision 55
# baseline (speedup 1.0000x reference)
"""Trainium2 Bass kernel for nn_BertSelfOutputPAL.

Data-parallel over batch: 8 batch elements -> 8 NeuronCores, no collectives.
Per core (batch element b), with S=2048, H=1024, P=256, T=4:
  h   = hs @ W + b                       (dense)
  tw  = softmax(h @ enc_W + mask)        (token gate over S)
  tv  = tw @ h
  td  = softmax(tv @ sel_W.T + sel_b)    (task gate over T)
  W_eff = sum_t td[t] * W1[t] @ W2[t]    (combined PAL weight, [H,H])
  x   = h + input + h @ W_eff (+ bias rows)
  out = LayerNorm(x) * g + beta

Key implementation choices vs a direct two-stage PAL evaluation:
  * The T adapters are collapsed into one effective [H,H] matrix on-device
    (td-weighted product of the down/up projections), cutting the PAL
    matmul work from 2*S*H*P*T to H*P*H*T + S*H*H MACs.
  * Dense runs in bf16 (same PE rate as fp32r but half the LDWEIGHTS
    cost); the W_eff build and its application run as fp8 (e4m3)
    DoubleRow matmuls at 2x PE rate. Weights are pre-scaled by 64 on the
    host so fp8 operands sit in the normal range; psums carry a 4096x
    scale that is divided out in the LayerNorm stage.
  * h is folded into the apply psum in natural layout via matmuls against
    a 4096-scaled identity (PE transpose at matched scale).
  * hs arrives host-transposed ([H, S]) so the dense matmul needs no
    on-device transposes. All activation I/O is bf16.
"""

import numpy as np
import ml_dtypes
from contextlib import ExitStack

import concourse.bacc as bacc
import concourse.bass as bass
import concourse.mybir as mybir
import concourse.tile as tile
from concourse.bass_utils import run_bass_kernel_spmd
from concourse.masks import make_identity

FP = mybir.dt.float32
BF = mybir.dt.bfloat16
F8 = mybir.dt.float8e4
AF = mybir.ActivationFunctionType
ALU = mybir.AluOpType
AX = mybir.AxisListType
DR = mybir.MatmulPerfMode.DoubleRow
EPS = 1e-12

B, S_FULL, H, P, T = 8, 2048, 1024, 256, 4
KT = H // 128       # 8 h-tiles
NP2 = KT // 2       # 4 paired h-tiles
N_CORES = 8
WSCALE = 64.0       # host pre-scale on W1/W2 fp8 payloads
PSC = WSCALE * WSCALE          # psum scale of fp8 products (4096)
IPSC = 1.0 / PSC

bf16 = ml_dtypes.bfloat16
f8e4 = ml_dtypes.float8_e4m3


def build_nc(S=S_FULL, zmask=True, zb1=True, zb2=True, zg=True, zb=True):
    SC = S // 512            # 512-wide s-chunks
    NST = S // 128           # 128-row s-tiles
    nc = bacc.Bacc("TRN2", target_bir_lowering=False, debug=False)

    # ---- DRAM I/O (per-core) ----
    # hsT/Wd arrive pre-tiled; hsT is chunk-major so each chunk DMA reads
    # 8KB-contiguous partition lines (fewer descriptors, full DMA rate)
    hsT_d = nc.dram_tensor("hsT", [SC, 128, KT, 512], BF, kind="ExternalInput").ap()
    inp_d = nc.dram_tensor("inp", [S, H], BF, kind="ExternalInput").ap()
    Wd_d = nc.dram_tensor("Wd", [128, KT, H], BF, kind="ExternalInput").ap()
    dbias_d = nc.dram_tensor("dbias", [128, KT], FP, kind="ExternalInput").ap()
    encw_d = nc.dram_tensor("encw", [128, KT], BF, kind="ExternalInput").ap()
    selw_d = nc.dram_tensor("selw", [128, KT, T], BF, kind="ExternalInput").ap()
    selb_d = nc.dram_tensor("selb", [1, T], BF, kind="ExternalInput").ap()
    w18_d = nc.dram_tensor("w18", [T, 128, 2, H], F8, kind="ExternalInput").ap()
    w28_d = nc.dram_tensor("w28", [T, 128, 2, H], F8, kind="ExternalInput").ap()
    mask_d = None if zmask else nc.dram_tensor("mask", [1, S], FP, kind="ExternalInput").ap()
    b18_d = None if zb1 else nc.dram_tensor("b18", [128, 2, T], F8, kind="ExternalInput").ap()
    b2_d = None if zb2 else nc.dram_tensor("b2", [T, H], BF, kind="ExternalInput").ap()
    lng_d = None if zg else nc.dram_tensor("lng", [1, H], FP, kind="ExternalInput").ap()
    lnb_d = None if zb else nc.dram_tensor("lnb", [1, H], FP, kind="ExternalInput").ap()
    outp = nc.dram_tensor("out", [S, H], BF, kind="ExternalOutput").ap()

    with tile.TileContext(nc) as tc, ExitStack() as ctx:
        # ---------- persistent pools ----------
        persist = ctx.enter_context(tc.tile_pool(name="persist", bufs=1))
        hp = ctx.enter_context(tc.tile_pool(name="hp", bufs=1))
        wp = ctx.enter_context(tc.tile_pool(name="wp", bufs=1))

        identf = persist.tile([128, 128], FP, tag="identf", name="identf")
        make_identity(nc, identf[:])
        id4096 = persist.tile([128, 128], BF, tag="id4096", name="id4096")
        nc.scalar.mul(id4096[:], identf[:], PSC)
        epst = persist.tile([128, 1], FP, tag="epst", name="epst")
        nc.gpsimd.memset(epst[:], EPS)
        zerot = persist.tile([128, 1], FP, tag="zerot", name="zerot")
        nc.gpsimd.memset(zerot[:], 0.0)
        ones1 = persist.tile([1, 128], BF, tag="ones1", name="ones1")
        nc.gpsimd.memset(ones1[:], 1.0)
        onev = persist.tile([1, 1], BF, tag="onev", name="onev")
        nc.gpsimd.memset(onev[:], 1.0)

        dbias = persist.tile([128, KT], FP, tag="dbias", name="dbias")
        encw = persist.tile([128, KT], BF, tag="encw", name="encw")
        selw = persist.tile([128, KT, T], BF, tag="selw", name="selw")
        selb = persist.tile([1, T], BF, tag="selb", name="selb")
        lngb = None if zg else persist.tile([128, H], FP, tag="lngb", name="lngb")
        lnbb = None if zb else persist.tile([128, H], FP, tag="lnbb", name="lnbb")

        twb32 = persist.tile([128, S], FP, tag="twb32", name="twb32")
        zrow = persist.tile([1, SC], FP, tag="zrow", name="zrow")
        tvp = persist.tile([128, KT, SC], FP, tag="tvp", name="tvp")
        tdrow = persist.tile([1, T], FP, tag="tdrow", name="tdrow")
        tdb = persist.tile([128, T], FP, tag="tdb", name="tdb")
        rz2b = persist.tile([128, 1], FP, tag="rz2b", name="rz2b")
        rrow = persist.tile([1, H], BF, tag="rrow", name="rrow")  # 4096x bias row

        # h in paired feature-major layout: [128, 2, S]; [:, i, :] = h-tile 2j+i
        htp = [hp.tile([128, 2, S], BF, tag=f"ht{j}", name=f"ht{j}") for j in range(NP2)]
        ht8p = [hp.tile([128, 2, S], F8, tag=f"h8{j}", name=f"h8{j}") for j in range(NP2)]
        weff8 = [wp.tile([128, 2, H], F8, tag=f"we{j}", name=f"we{j}") for j in range(NP2)]
        w18s = [wp.tile([128, 2, H], F8, tag=f"w1_{t}", name=f"w1_{t}") for t in range(T)]
        w28s = [wp.tile([128, 2, H], F8, tag=f"w2_{t}", name=f"w2_{t}") for t in range(T)]
        # ================= pass A: dense + logits + streaming softmax/tv ====
        # Token-gate softmax runs unnormalized (logits are O(1); exp is safe
        # in fp32 without max subtraction), so exp/broadcast/tv-accumulate
        # pipeline per 512-chunk under the dense matmuls.
        with tc.tile_pool(name="pA", bufs=1) as pa, \
             tc.tile_pool(name="pA_ps_d", bufs=3, space="PSUM") as dps, \
             tc.tile_pool(name="pA_ps_l", bufs=2, space="PSUM") as lps_p:
            # Wd on the scalar (Activation) DGE queue, hsT on the sync queue:
            # the two transfers overlap, so dense starts after ~max of the two
            # Wd split 3 ways (sync/scalar/gpsimd queues), hsT chunks alternate
            wd_sb = pa.tile([128, KT, H], BF, tag="wd", name="wd")
            nc.sync.dma_start(wd_sb[:, 0:3, :], Wd_d[:, 0:3, :])
            nc.scalar.dma_start(wd_sb[:, 3:6, :], Wd_d[:, 3:6, :])
            nc.gpsimd.dma_start(wd_sb[:, 6:8, :], Wd_d[:, 6:8, :])
            hst = pa.tile([128, KT, S], BF, tag="hst", name="hst")
            for c in range(SC):
                cw = slice(c * 512, (c + 1) * 512)
                eng = nc.sync if c % 2 == 0 else nc.scalar
                eng.dma_start(hst[:, :, cw], hsT_d[c])
            nc.gpsimd.dma_start(dbias[:], dbias_d)
            nc.gpsimd.dma_start(encw[:], encw_d)
            nc.gpsimd.dma_start(selw[:], selw_d)
            nc.gpsimd.dma_start(selb[:], selb_d)
            for t in range(T):
                nc.scalar.dma_start(w18s[t][:], w18_d[t])
                nc.scalar.dma_start(w28s[t][:], w28_d[t])
            if not zmask:
                mrow = pa.tile([1, S], FP, tag="mrow", name="mrow")
                nc.sync.dma_start(mrow[:], mask_d)
            erow = pa.tile([1, S], FP, tag="erow", name="erow")
            scr = pa.tile([128, 512], FP, tag="scr", name="scr")

            for c in range(SC):
                cw = slice(c * 512, (c + 1) * 512)
                for mt in range(KT):
                    ps = dps.tile([128, 512], FP, tag="dps", name="dps")
                    for kt in range(KT):
                        nc.tensor.matmul(
                            ps[:],
                            wd_sb[:, kt, mt * 128:(mt + 1) * 128],
                            hst[:, kt, cw],
                            start=(kt == 0), stop=(kt == KT - 1),
                        )
                    nc.scalar.activation(
                        htp[mt // 2][:, mt % 2, cw], ps[:], AF.Identity,
                        bias=dbias[:, mt:mt + 1], scale=1.0)
                # token-gate logits for this chunk
                lps = lps_p.tile([1, 512], FP, tag="lps", name="lps")
                for kt in range(KT):
                    nc.tensor.matmul(
                        lps[:], encw[:, kt:kt + 1], htp[kt // 2][:, kt % 2, cw],
                        start=(kt == 0), stop=(kt == KT - 1),
                    )
                # unnormalized exp + running Z, then tv partials on DVE
                if zmask:
                    nc.scalar.activation(erow[:, cw], lps[:], AF.Exp,
                                         bias=0.0, scale=1.0,
                                         accum_out=zrow[:, c:c + 1])
                else:
                    nc.vector.tensor_add(erow[:, cw], lps[:], mrow[:, cw])
                    nc.scalar.activation(erow[:, cw], erow[:, cw], AF.Exp,
                                         bias=0.0, scale=1.0,
                                         accum_out=zrow[:, c:c + 1])
                nc.gpsimd.partition_broadcast(twb32[:, cw], erow[:, cw])
                for kt in range(KT):
                    nc.vector.scalar_tensor_tensor(
                        scr[:], htp[kt // 2][:, kt % 2, cw], 1.0, twb32[:, cw],
                        op0=ALU.mult, op1=ALU.mult,
                        accum_out=tvp[:, kt:kt + 1, c])
                # fp8 h copies from the bf16 h (after bc so the broadcast isn't
                # stuck behind them on the gpsimd queue)
                for mt in range(KT):
                    eng8 = nc.vector if mt % 2 == 0 else nc.gpsimd
                    eng8.tensor_copy(ht8p[mt // 2][:, mt % 2, cw],
                                     htp[mt // 2][:, mt % 2, cw])

        # ================= pass B: finish gates, td, scale W1 ================
        with tc.tile_pool(name="pB", bufs=1) as pb, \
             tc.tile_pool(name="pB_ps", bufs=2, space="PSUM") as bps:
            if not zg:
                lngr = pb.tile([1, H], FP, tag="lngr", name="lngr")
                nc.sync.dma_start(lngr[:], lng_d)
                nc.gpsimd.partition_broadcast(lngb[:], lngr[:])
            if not zb:
                lnbr = pb.tile([1, H], FP, tag="lnbr", name="lnbr")
                nc.sync.dma_start(lnbr[:], lnb_d)
                nc.gpsimd.partition_broadcast(lnbb[:], lnbr[:])
            # tv = (sum_c tv_partial[c]) / Z  (Z folds into tvcols)
            zsum = pb.tile([1, 1], FP, tag="zsum", name="zsum")
            nc.vector.reduce_sum(zsum[:], zrow[:], axis=AX.X)
            rz = pb.tile([1, 1], FP, tag="rz", name="rz")
            nc.vector.reciprocal(rz[:], zsum[:])
            rzb = pb.tile([128, 1], FP, tag="rzb", name="rzb")
            nc.gpsimd.partition_broadcast(rzb[:], rz[:])
            tva = pb.tile([128, KT], FP, tag="tva", name="tva")
            nc.vector.reduce_sum(tva[:], tvp[:], axis=AX.X)
            tvcols = pb.tile([128, KT], BF, tag="tvcols", name="tvcols")
            nc.vector.tensor_scalar(tvcols[:], tva[:], rzb[:], None, op0=ALU.mult)
            # td logits = tv @ selW.T + selb  -> [1, T]; exp straight off PSUM
            # (logits are O(0.1), no max subtraction needed)
            ps = bps.tile([1, T], FP, tag="bmisc", name="bmisc")
            for kt in range(KT):
                nc.tensor.matmul(ps[:], tvcols[:, kt:kt + 1], selw[:, kt, :],
                                 start=(kt == 0), stop=False)
            nc.tensor.matmul(ps[:], onev[:], selb[:], start=False, stop=True)
            tdl = pb.tile([1, T], FP, tag="tdl", name="tdl")
            z2 = pb.tile([1, 1], FP, tag="z2", name="z2")
            nc.scalar.activation(tdl[:], ps[:], AF.Exp, bias=0.0, scale=1.0,
                                 accum_out=z2[:])
            # scale W1 by UNNORMALIZED exp(td logits); the 1/Z softmax
            # normalization folds into the W_eff psum->fp8 copies (off the
            # critical path to the W_eff matmuls)
            nc.gpsimd.partition_broadcast(tdb[:], tdl[:])
            for t in range(T):
                nc.vector.tensor_scalar(w18s[t][:], w18s[t][:], tdb[:, t:t + 1],
                                        None, op0=ALU.mult)
            rz2 = pb.tile([1, 1], FP, tag="rz2", name="rz2")
            nc.vector.reciprocal(rz2[:], z2[:])
            nc.gpsimd.partition_broadcast(rz2b[:], rz2[:])
            if not (zb1 and zb2):
                nc.vector.tensor_scalar(tdrow[:], tdl[:], rz2[:], None,
                                        op0=ALU.mult)

            # optional bias rows (4096x domain): r = sum_t td[t]*(b1[t]@W2[t] + b2[t])
            if not (zb1 and zb2):
                rps = [bps.tile([1, 512], FP, tag="bmisc", name="bmisc")
                       for _ in range(2)]
                tdbn = pb.tile([128, T], FP, tag="tdbn", name="tdbn")
                nc.gpsimd.partition_broadcast(tdbn[:], tdrow[:])
                if not zb1:
                    b18 = pb.tile([128, 2, T], F8, tag="b18", name="b18")
                    nc.sync.dma_start(b18[:], b18_d)
                    for t in range(T):
                        nc.vector.tensor_scalar(b18[:, :, t], b18[:, :, t],
                                                tdbn[:, t:t + 1], None, op0=ALU.mult)
                for hc in range(2):
                    first = True
                    if not zb1:
                        for t in range(T):
                            for q in range(2):
                                nc.tensor.matmul(
                                    rps[hc][:, q * 256:(q + 1) * 256],
                                    b18[:, :, t:t + 1],
                                    w28s[t][:, :, (hc * 2 + q) * 256:(hc * 2 + q + 1) * 256],
                                    start=first and q == 0, stop=False,
                                    perf_mode=DR)
                            first = False
                    if not zb2:
                        b2n = pb.tile([T, H], BF, tag="b2n", name="b2n")
                        if hc == 0:
                            nc.sync.dma_start(b2n[:], b2_d)
                            tdc = pb.tile([T, 2], FP, tag="tdc", name="tdc")
                            tdcb = pb.tile([T, 1], BF, tag="tdcb", name="tdcb")
                            nc.tensor.matmul(tdc[:], tdrow[:].bitcast(FP), ones1[:, :2].bitcast(BF),
                                             start=True, stop=True, skip_group_check=True)
                        # td as column via PSUM transpose trick happens above;
                        # r2 = td @ b2 scaled to the 4096x domain
                        nc.scalar.mul(tdcb[:], tdc[:, :1], PSC)
                        nc.tensor.matmul(rps[hc][:], tdcb[:],
                                         b2n[:, hc * 512:(hc + 1) * 512],
                                         start=zb1, stop=True)
                    else:
                        # close the accumulation group opened by the b1 matmuls
                        nc.tensor.matmul(rps[hc][:, 256:512], b18[:, :, T - 1:T],
                                         w28s[T - 1][:, :, (hc * 2 + 1) * 256:(hc * 2 + 2) * 256],
                                         start=False, stop=True, perf_mode=DR)
                    nc.scalar.copy(rrow[:, hc * 512:(hc + 1) * 512], rps[hc][:])

        # ================= pass C1: W_eff build (fp8 DoubleRow) ==============
        with tc.tile_pool(name="pC1_ps", bufs=8, space="PSUM") as weps:
            for hc in range(2):
                pss = [weps.tile([128, 512], FP, tag="weps", name="weps")
                       for mt in range(KT)]
                for t in range(T):
                    for mt in range(KT):
                        nc.tensor.matmul(
                            pss[mt][:],
                            w18s[t][:, :, mt * 128:(mt + 1) * 128],
                            w28s[t][:, :, hc * 512:(hc + 1) * 512],
                            start=(t == 0), stop=(t == T - 1),
                            perf_mode=DR)
                for mt in range(KT):
                    dst = weff8[mt // 2][:, mt % 2, hc * 512:(hc + 1) * 512]
                    if mt % 2 == 0:
                        nc.vector.tensor_scalar(dst, pss[mt][:], rz2b[:], None,
                                                op0=ALU.mult)
                    else:
                        nc.scalar.activation(dst, pss[mt][:], AF.Identity,
                                             bias=zerot[:], scale=rz2b[:])

        # ================= pass C2: apply + residual + LayerNorm =============
        aps_p = ctx.enter_context(tc.tile_pool(name="pC2_ps", bufs=8, space="PSUM"))
        in_pool = ctx.enter_context(tc.tile_pool(name="in3", bufs=6))
        xt_pool = ctx.enter_context(tc.tile_pool(name="xt3", bufs=4))
        xo_pool = ctx.enter_context(tc.tile_pool(name="xo3", bufs=4))
        sq_pool = ctx.enter_context(tc.tile_pool(name="sq3", bufs=4))
        stats = ctx.enter_context(tc.tile_pool(name="stats", bufs=8))

        in_tiles = {}

        def load_inp(st):
            it = in_pool.tile([128, H], BF, tag="inp", name="inp")
            nc.sync.dma_start(it[:], inp_d[st * 128:(st + 1) * 128, :])
            in_tiles[st] = it

        for st in range(5):
            load_inp(st)

        have_r = not (zb1 and zb2)
        for st in range(NST):
            if st + 5 < NST:
                load_inp(st + 5)
            sw = slice(st * 128, (st + 1) * 128)
            it = in_tiles.pop(st)
            pss = []
            for hc in range(2):
                ps = aps_p.tile([128, 512], FP, tag="xps", name="xps")
                # 4096*input via PE against the scaled identity (opens group)
                nc.tensor.matmul(ps[:], id4096[:],
                                 it[:, hc * 512:(hc + 1) * 512],
                                 start=True, stop=False)
                for j in range(4):
                    kt = hc * 4 + j
                    nc.tensor.matmul(
                        ps[:, j * 128:(j + 1) * 128],
                        htp[kt // 2][:, kt % 2, sw], id4096[:],
                        start=False, stop=False)
                for j in range(NP2):
                    nc.tensor.matmul(
                        ps[:],
                        ht8p[j][:, :, sw],
                        weff8[j][:, :, hc * 512:(hc + 1) * 512],
                        start=False, stop=(j == NP2 - 1 and not have_r),
                        perf_mode=DR)
                if have_r:
                    nc.tensor.matmul(ps[:], ones1[:],
                                     rrow[:, hc * 512:(hc + 1) * 512],
                                     start=False, stop=True)
                pss.append(ps)
            # ---- LayerNorm (x sits in psum at 4096x scale) ----
            xt = xt_pool.tile([128, H], BF, tag="x", name="x")
            s0 = stats.tile([128, 1], FP, tag="s0", name="s0")
            s1 = stats.tile([128, 1], FP, tag="s1", name="s1")
            nc.vector.tensor_scalar(xt[:, 0:512], pss[0][:], IPSC, 0.0,
                                    op0=ALU.mult, op1=ALU.add, accum_out=s0[:])
            nc.scalar.activation(xt[:, 512:1024], pss[1][:], AF.Identity,
                                 bias=zerot[:], scale=IPSC, accum_out=s1[:])
            sq = sq_pool.tile([128, H], BF, tag="sq", name="sq")
            sqa = stats.tile([128, 1], FP, tag="sqa", name="sqa")
            sqb = stats.tile([128, 1], FP, tag="sqb", name="sqb")
            nc.scalar.activation(sq[:, 0:512], xt[:, 0:512], AF.Square,
                                 bias=zerot[:], accum_out=sqa[:])
            nc.vector.scalar_tensor_tensor(sq[:, 512:1024], xt[:, 512:1024], 1.0,
                                           xt[:, 512:1024], op0=ALU.mult,
                                           op1=ALU.mult, accum_out=sqb[:])
            # var = (ssq - ssum^2/H)/H; mu = ssum/H
            ssum = stats.tile([128, 1], FP, tag="ssum", name="ssum")
            nc.vector.tensor_add(ssum[:], s0[:], s1[:])
            q_ = stats.tile([128, 1], FP, tag="q", name="q")
            nc.vector.scalar_tensor_tensor(q_[:], ssum[:], 1.0 / H, ssum[:],
                                           op0=ALU.mult, op1=ALU.mult)
            mu = stats.tile([128, 1], FP, tag="mu", name="mu")
            nc.vector.tensor_scalar(mu[:], ssum[:], 1.0 / H, None, op0=ALU.mult)
            v_ = stats.tile([128, 1], FP, tag="v", name="v")
            nc.vector.scalar_tensor_tensor(v_[:], q_[:], -1.0, sqa[:],
                                           op0=ALU.mult, op1=ALU.add)
            v2_ = stats.tile([128, 1], FP, tag="v2", name="v2")
            nc.vector.tensor_add(v2_[:], v_[:], sqb[:])
            sd = stats.tile([128, 1], FP, tag="sd", name="sd")
            nc.scalar.activation(sd[:], v2_[:], AF.Sqrt, bias=epst[:], scale=1.0 / H)
            isd = stats.tile([128, 1], FP, tag="isd", name="isd")
            nc.vector.reciprocal(isd[:], sd[:])
            xo = xo_pool.tile([128, H], BF, tag="xo", name="xo")
            if zg and zb:
                # xnorm alternates engines to balance DVE vs scalar load
                if st % 2 == 0:
                    nc.vector.tensor_scalar(xo[:], xt[:], mu[:], isd[:],
                                            op0=ALU.subtract, op1=ALU.mult)
                else:
                    nmi = stats.tile([128, 1], FP, tag="nmi", name="nmi")
                    nc.vector.scalar_tensor_tensor(nmi[:], mu[:], -1.0, isd[:],
                                                   op0=ALU.mult, op1=ALU.mult)
                    nc.scalar.activation(xo[:], xt[:], AF.Identity,
                                         bias=nmi[:], scale=isd[:])
            else:
                nc.vector.tensor_scalar(xt[:], xt[:], mu[:], isd[:],
                                        op0=ALU.subtract, op1=ALU.mult)
                if not zg:
                    nc.vector.scalar_tensor_tensor(xt[:], xt[:], 1.0, lngb[:],
                                                   op0=ALU.mult, op1=ALU.mult)
                if not zb:
                    nc.gpsimd.tensor_add(xt[:], xt[:], lnbb[:])
                nc.scalar.copy(xo[:], xt[:])
            nc.sync.dma_start(outp[sw, :], xo[:])

    nc.finalize()
    return nc


_CACHE = {}


def _get_nc(S=S_FULL, zmask=True, zb1=True, zb2=True, zg=True, zb=True):
    key = (S, zmask, zb1, zb2, zg, zb)
    if key not in _CACHE:
        _CACHE[key] = build_nc(S, zmask=zmask, zb1=zb1, zb2=zb2, zg=zg, zb=zb)
    return _CACHE[key]


def _flags(inputs):
    f32 = lambda x: np.asarray(x, dtype=np.float32)
    return dict(
        zmask=not np.any(f32(inputs["attention_mask"])),
        zb1=not np.any(f32(inputs["pal_b1"])),
        zb2=not np.any(f32(inputs["pal_b2"])),
        zg=bool(np.all(f32(inputs["ln_g"]) == 1.0)),
        zb=not np.any(f32(inputs["ln_b"])),
    )


def _in_maps(inputs, S=S_FULL):
    f32 = lambda x: np.ascontiguousarray(np.asarray(x), dtype=np.float32)
    flags = _flags(inputs)
    hs = f32(inputs["hidden_states"])
    inp = f32(inputs["input_tensor"])
    msk = f32(inputs["attention_mask"]).reshape(B, S)
    Wd = np.ascontiguousarray(
        f32(inputs["dense_W"]).reshape(KT, 128, H).transpose(1, 0, 2)).astype(bf16)
    dbias = f32(inputs["dense_b"]).reshape(KT, 128).T.copy()
    W1 = f32(inputs["pal_W1"])   # [T, H, P]
    W2 = f32(inputs["pal_W2"])   # [T, P, H]
    w18 = np.ascontiguousarray(
        (W1.transpose(0, 2, 1) * WSCALE).reshape(T, 2, 128, H).transpose(0, 2, 1, 3)
    ).astype(f8e4)
    w28 = np.ascontiguousarray(
        (W2 * WSCALE).reshape(T, 2, 128, H).transpose(0, 2, 1, 3)
    ).astype(f8e4)
    encw = f32(inputs["enc_W"]).reshape(KT, 128).T.astype(bf16)
    selw = np.ascontiguousarray(
        f32(inputs["sel_W"]).reshape(T, KT, 128).transpose(2, 1, 0)).astype(bf16)
    selb = f32(inputs["sel_b"]).reshape(1, T).astype(bf16)
    shared = dict(Wd=Wd, dbias=dbias, w18=w18, w28=w28, encw=encw,
                  selw=selw, selb=selb)
    if not flags["zb1"]:
        shared["b18"] = np.ascontiguousarray(
            (f32(inputs["pal_b1"]) * WSCALE).reshape(T, 2, 128).transpose(2, 1, 0)
        ).astype(f8e4)
    if not flags["zb2"]:
        shared["b2"] = f32(inputs["pal_b2"]).astype(bf16)
    if not flags["zg"]:
        shared["lng"] = f32(inputs["ln_g"]).reshape(1, H)
    if not flags["zb"]:
        shared["lnb"] = f32(inputs["ln_b"]).reshape(1, H)
    maps = []
    for b in range(B):
        m = dict(hsT=np.ascontiguousarray(
                     hs[b].T.reshape(KT, 128, S // 512, 512)
                     .transpose(2, 1, 0, 3)).astype(bf16),
                 inp=inp[b].astype(bf16), **shared)
        if not flags["zmask"]:
            m["mask"] = msk[b:b + 1]
        maps.append(m)
    return maps


def kernel(**inputs):
    nc = _get_nc(**_flags(inputs))
    res = run_bass_kernel_spmd(nc, _in_maps(inputs), list(range(N_CORES)))
    out = np.stack([np.asarray(res.results[b]["out"], dtype=np.float32)
                    for b in range(B)], axis=0)
    return out


# revision 60
# speedup vs baseline: 1.0275x; 1.0275x over previous
"""Trainium2 Bass kernel for nn_BertSelfOutputPAL.

Data-parallel over batch: 8 batch elements -> 8 NeuronCores, no collectives.
Per core (batch element b), with S=2048, H=1024, P=256, T=4:
  h   = hs @ W + b                       (dense)
  tw  = softmax(h @ enc_W + mask)        (token gate over S)
  tv  = tw @ h
  td  = softmax(tv @ sel_W.T + sel_b)    (task gate over T)
  W_eff = sum_t td[t] * W1[t] @ W2[t]    (combined PAL weight, [H,H])
  x   = h + input + h @ W_eff (+ bias rows)
  out = LayerNorm(x) * g + beta

Key implementation choices vs a direct two-stage PAL evaluation:
  * The T adapters are collapsed into one effective [H,H] matrix on-device
    (td-weighted product of the down/up projections), cutting the PAL
    matmul work from 2*S*H*P*T to H*P*H*T + S*H*H MACs.
  * Dense runs in bf16 (same PE rate as fp32r but half the LDWEIGHTS
    cost); the W_eff build and its application run as fp8 (e4m3)
    DoubleRow matmuls at 2x PE rate. Weights are pre-scaled by 64 on the
    host so fp8 operands sit in the normal range; psums carry a 4096x
    scale that is divided out in the LayerNorm stage.
  * h is folded into the apply psum in natural layout via matmuls against
    a 4096-scaled identity (PE transpose at matched scale).
  * hs arrives host-transposed ([H, S]) so the dense matmul needs no
    on-device transposes. All activation I/O is bf16.
"""

import numpy as np
import ml_dtypes
from contextlib import ExitStack

import concourse.bacc as bacc
import concourse.bass as bass
import concourse.mybir as mybir
import concourse.tile as tile
from concourse.bass_utils import run_bass_kernel_spmd
from concourse.masks import make_identity

FP = mybir.dt.float32
BF = mybir.dt.bfloat16
F8 = mybir.dt.float8e4
AF = mybir.ActivationFunctionType
ALU = mybir.AluOpType
AX = mybir.AxisListType
DR = mybir.MatmulPerfMode.DoubleRow
EPS = 1e-12

B, S_FULL, H, P, T = 8, 2048, 1024, 256, 4
KT = H // 128       # 8 h-tiles
NP2 = KT // 2       # 4 paired h-tiles
N_CORES = 8
WSCALE = 64.0       # host pre-scale on W1/W2 fp8 payloads
PSC = WSCALE * WSCALE          # psum scale of fp8 products (4096)
IPSC = 1.0 / PSC

bf16 = ml_dtypes.bfloat16
f8e4 = ml_dtypes.float8_e4m3


def build_nc(S=S_FULL, zmask=True, zb1=True, zb2=True, zg=True, zb=True):
    SC = S // 512            # 512-wide s-chunks
    NST = S // 128           # 128-row s-tiles
    nc = bacc.Bacc("TRN2", target_bir_lowering=False, debug=False)

    # ---- DRAM I/O (per-core) ----
    # hsT/Wd arrive pre-tiled; hsT is chunk-major so each chunk DMA reads
    # 8KB-contiguous partition lines (fewer descriptors, full DMA rate)
    hsT_d = nc.dram_tensor("hsT", [SC, 128, KT, 512], BF, kind="ExternalInput").ap()
    inp_d = nc.dram_tensor("inp", [S, H], BF, kind="ExternalInput").ap()
    Wd_d = nc.dram_tensor("Wd", [128, KT, H], BF, kind="ExternalInput").ap()
    dbias_d = nc.dram_tensor("dbias", [128, KT], FP, kind="ExternalInput").ap()
    encw_d = nc.dram_tensor("encw", [128, KT], BF, kind="ExternalInput").ap()
    selw_d = nc.dram_tensor("selw", [128, KT, T], BF, kind="ExternalInput").ap()
    selb_d = nc.dram_tensor("selb", [1, T], BF, kind="ExternalInput").ap()
    w18_d = nc.dram_tensor("w18", [T, 128, 2, H], F8, kind="ExternalInput").ap()
    w28_d = nc.dram_tensor("w28", [T, 128, 2, H], F8, kind="ExternalInput").ap()
    mask_d = None if zmask else nc.dram_tensor("mask", [1, S], FP, kind="ExternalInput").ap()
    b18_d = None if zb1 else nc.dram_tensor("b18", [128, 2, T], F8, kind="ExternalInput").ap()
    b2_d = None if zb2 else nc.dram_tensor("b2", [T, H], BF, kind="ExternalInput").ap()
    lng_d = None if zg else nc.dram_tensor("lng", [1, H], FP, kind="ExternalInput").ap()
    lnb_d = None if zb else nc.dram_tensor("lnb", [1, H], FP, kind="ExternalInput").ap()
    outp = nc.dram_tensor("out", [S, H], BF, kind="ExternalOutput").ap()

    with tile.TileContext(nc) as tc, ExitStack() as ctx:
        # ---------- persistent pools ----------
        persist = ctx.enter_context(tc.tile_pool(name="persist", bufs=1))
        hp = ctx.enter_context(tc.tile_pool(name="hp", bufs=1))
        wp = ctx.enter_context(tc.tile_pool(name="wp", bufs=1))

        identf = persist.tile([128, 128], FP, tag="identf", name="identf")
        make_identity(nc, identf[:])
        id4096 = persist.tile([128, 128], BF, tag="id4096", name="id4096")
        nc.scalar.mul(id4096[:], identf[:], PSC)
        epst = persist.tile([128, 1], FP, tag="epst", name="epst")
        nc.gpsimd.memset(epst[:], EPS)
        zerot = persist.tile([128, 1], FP, tag="zerot", name="zerot")
        nc.gpsimd.memset(zerot[:], 0.0)
        ones1 = persist.tile([1, 128], BF, tag="ones1", name="ones1")
        nc.gpsimd.memset(ones1[:], 1.0)
        onev = persist.tile([1, 1], BF, tag="onev", name="onev")
        nc.gpsimd.memset(onev[:], 1.0)

        dbias = persist.tile([128, KT], FP, tag="dbias", name="dbias")
        encw = persist.tile([128, KT], BF, tag="encw", name="encw")
        selw = persist.tile([128, KT, T], BF, tag="selw", name="selw")
        selb = persist.tile([1, T], BF, tag="selb", name="selb")
        lngb = None if zg else persist.tile([128, H], FP, tag="lngb", name="lngb")
        lnbb = None if zb else persist.tile([128, H], FP, tag="lnbb", name="lnbb")

        twb4 = persist.tile([4, S], FP, tag="twb4", name="twb4")
        zrow = persist.tile([1, SC], FP, tag="zrow", name="zrow")
        tdp = persist.tile([4, SC], FP, tag="tdp", name="tdp")
        tdrow = persist.tile([1, T], FP, tag="tdrow", name="tdrow")
        tdb = persist.tile([128, T], FP, tag="tdb", name="tdb")
        rz2b = persist.tile([128, 1], FP, tag="rz2b", name="rz2b")
        rrow = persist.tile([1, H], BF, tag="rrow", name="rrow")  # 4096x bias row

        # h in paired feature-major layout: [128, 2, S]; [:, i, :] = h-tile 2j+i
        htp = [hp.tile([128, 2, S], BF, tag=f"ht{j}", name=f"ht{j}") for j in range(NP2)]
        ht8p = [hp.tile([128, 2, S], F8, tag=f"h8{j}", name=f"h8{j}") for j in range(NP2)]
        weff8 = [wp.tile([128, 2, H], F8, tag=f"we{j}", name=f"we{j}") for j in range(NP2)]
        w18s = [wp.tile([128, 2, H], F8, tag=f"w1_{t}", name=f"w1_{t}") for t in range(T)]
        w28s = [wp.tile([128, 2, H], F8, tag=f"w2_{t}", name=f"w2_{t}") for t in range(T)]
        # ================= pass A: dense + logits + streaming softmax/tv ====
        # Token-gate softmax runs unnormalized (logits are O(1); exp is safe
        # in fp32 without max subtraction), so exp/broadcast/tv-accumulate
        # pipeline per 512-chunk under the dense matmuls.
        with tc.tile_pool(name="pA", bufs=1) as pa, \
             tc.tile_pool(name="pA_ps_d", bufs=3, space="PSUM") as dps, \
             tc.tile_pool(name="pA_ps_g", bufs=2, space="PSUM") as gps_p, \
             tc.tile_pool(name="pA_ps_l", bufs=2, space="PSUM") as lps_p:
            # Wd on the scalar (Activation) DGE queue, hsT on the sync queue:
            # the two transfers overlap, so dense starts after ~max of the two
            # Wd split 3 ways (sync/scalar/gpsimd queues), hsT chunks alternate
            wd_sb = pa.tile([128, KT, H], BF, tag="wd", name="wd")
            nc.sync.dma_start(wd_sb[:, 0:3, :], Wd_d[:, 0:3, :])
            nc.scalar.dma_start(wd_sb[:, 3:6, :], Wd_d[:, 3:6, :])
            nc.gpsimd.dma_start(wd_sb[:, 6:8, :], Wd_d[:, 6:8, :])
            hst = pa.tile([128, KT, S], BF, tag="hst", name="hst")
            for c in range(SC):
                cw = slice(c * 512, (c + 1) * 512)
                eng = nc.sync if c % 2 == 0 else nc.scalar
                eng.dma_start(hst[:, :, cw], hsT_d[c])
            nc.gpsimd.dma_start(dbias[:], dbias_d)
            nc.gpsimd.dma_start(encw[:], encw_d)
            nc.gpsimd.dma_start(selw[:], selw_d)
            nc.gpsimd.dma_start(selb[:], selb_d)
            for t in range(T):
                nc.scalar.dma_start(w18s[t][:], w18_d[t])
                nc.scalar.dma_start(w28s[t][:], w28_d[t])
            if not zmask:
                mrow = pa.tile([1, S], FP, tag="mrow", name="mrow")
                nc.sync.dma_start(mrow[:], mask_d)
            erow = pa.tile([1, S], FP, tag="erow", name="erow")
            scr4 = pa.tile([4, 512], FP, tag="scr4", name="scr4")

            for c in range(SC):
                cw = slice(c * 512, (c + 1) * 512)
                for mt in range(KT):
                    ps = dps.tile([128, 512], FP, tag="dps", name="dps")
                    for kt in range(KT):
                        nc.tensor.matmul(
                            ps[:],
                            wd_sb[:, kt, mt * 128:(mt + 1) * 128],
                            hst[:, kt, cw],
                            start=(kt == 0), stop=(kt == KT - 1),
                        )
                    nc.scalar.activation(
                        htp[mt // 2][:, mt % 2, cw], ps[:], AF.Identity,
                        bias=dbias[:, mt:mt + 1], scale=1.0)
                # token-gate logits for this chunk
                lps = lps_p.tile([1, 512], FP, tag="lps", name="lps")
                for kt in range(KT):
                    nc.tensor.matmul(
                        lps[:], encw[:, kt:kt + 1], htp[kt // 2][:, kt % 2, cw],
                        start=(kt == 0), stop=(kt == KT - 1),
                    )
                # unnormalized exp + running Z
                if zmask:
                    nc.scalar.activation(erow[:, cw], lps[:], AF.Exp,
                                         bias=0.0, scale=1.0,
                                         accum_out=zrow[:, c:c + 1])
                else:
                    nc.vector.tensor_add(erow[:, cw], lps[:], mrow[:, cw])
                    nc.scalar.activation(erow[:, cw], erow[:, cw], AF.Exp,
                                         bias=0.0, scale=1.0,
                                         accum_out=zrow[:, c:c + 1])
                nc.gpsimd.partition_broadcast(twb4[:, cw], erow[:, cw])
                # task-gate partials: td_logits[t] = sum_s tw[s]*(h[s]@selW[t])
                # via g = selW @ h^T on PE, then a tiny [4,512] DVE reduce
                gps = gps_p.tile([4, 512], FP, tag="gps", name="gps")
                for kt in range(KT):
                    nc.tensor.matmul(
                        gps[:], selw[:, kt, :], htp[kt // 2][:, kt % 2, cw],
                        start=(kt == 0), stop=(kt == KT - 1),
                    )
                nc.vector.scalar_tensor_tensor(
                    scr4[:], gps[:], 1.0, twb4[:, cw],
                    op0=ALU.mult, op1=ALU.mult, accum_out=tdp[:, c:c + 1])
                # fp8 h copies from the bf16 h
                for mt in range(KT):
                    eng8 = nc.vector if mt % 2 == 0 else nc.gpsimd
                    eng8.tensor_copy(ht8p[mt // 2][:, mt % 2, cw],
                                     htp[mt // 2][:, mt % 2, cw])

        # ================= pass B: finish gates, td, scale W1 ================
        with tc.tile_pool(name="pB", bufs=1) as pb, \
             tc.tile_pool(name="pB_ps", bufs=2, space="PSUM") as bps:
            if not zg:
                lngr = pb.tile([1, H], FP, tag="lngr", name="lngr")
                nc.sync.dma_start(lngr[:], lng_d)
                nc.gpsimd.partition_broadcast(lngb[:], lngr[:])
            if not zb:
                lnbr = pb.tile([1, H], FP, tag="lnbr", name="lnbr")
                nc.sync.dma_start(lnbr[:], lnb_d)
                nc.gpsimd.partition_broadcast(lnbb[:], lnbr[:])
            # td logits (unnormalized-by-Z) = sum_c tdp[:, c]; transpose the
            # [4,1] column to a [1,4] row on PE, scale by 1/Z, add selb, exp.
            zsum = pb.tile([1, 1], FP, tag="zsum", name="zsum")
            nc.vector.reduce_sum(zsum[:], zrow[:], axis=AX.X)
            rz = pb.tile([1, 1], FP, tag="rz", name="rz")
            nc.vector.reciprocal(rz[:], zsum[:])
            tdsum = pb.tile([4, 1], FP, tag="tdsum", name="tdsum")
            nc.vector.reduce_sum(tdsum[:], tdp[:], axis=AX.X)
            ps = bps.tile([1, T], FP, tag="bmisc", name="bmisc")
            nc.tensor.matmul(ps[:], tdsum[:], identf[0:4, 0:4],
                             start=True, stop=True)
            tdl_a = pb.tile([1, T], FP, tag="tdl_a", name="tdl_a")
            nc.scalar.activation(tdl_a[:], ps[:], AF.Identity,
                                 bias=zerot[0:1, :], scale=rz[:])
            tdl2 = pb.tile([1, T], FP, tag="tdl2", name="tdl2")
            nc.vector.tensor_add(tdl2[:], tdl_a[:], selb[:])
            tdl = pb.tile([1, T], FP, tag="tdl", name="tdl")
            z2 = pb.tile([1, 1], FP, tag="z2", name="z2")
            nc.scalar.activation(tdl[:], tdl2[:], AF.Exp, bias=0.0, scale=1.0,
                                 accum_out=z2[:])
            # scale W1 by UNNORMALIZED exp(td logits); the 1/Z softmax
            # normalization folds into the W_eff psum->fp8 copies (off the
            # critical path to the W_eff matmuls)
            nc.gpsimd.partition_broadcast(tdb[:], tdl[:])
            for t in range(T):
                nc.vector.tensor_scalar(w18s[t][:], w18s[t][:], tdb[:, t:t + 1],
                                        None, op0=ALU.mult)
            rz2 = pb.tile([1, 1], FP, tag="rz2", name="rz2")
            nc.vector.reciprocal(rz2[:], z2[:])
            nc.gpsimd.partition_broadcast(rz2b[:], rz2[:])
            if not (zb1 and zb2):
                nc.vector.tensor_scalar(tdrow[:], tdl[:], rz2[:], None,
                                        op0=ALU.mult)

            # optional bias rows (4096x domain): r = sum_t td[t]*(b1[t]@W2[t] + b2[t])
            if not (zb1 and zb2):
                rps = [bps.tile([1, 512], FP, tag="bmisc", name="bmisc")
                       for _ in range(2)]
                tdbn = pb.tile([128, T], FP, tag="tdbn", name="tdbn")
                nc.gpsimd.partition_broadcast(tdbn[:], tdrow[:])
                if not zb1:
                    b18 = pb.tile([128, 2, T], F8, tag="b18", name="b18")
                    nc.sync.dma_start(b18[:], b18_d)
                    for t in range(T):
                        nc.vector.tensor_scalar(b18[:, :, t], b18[:, :, t],
                                                tdbn[:, t:t + 1], None, op0=ALU.mult)
                for hc in range(2):
                    first = True
                    if not zb1:
                        for t in range(T):
                            for q in range(2):
                                nc.tensor.matmul(
                                    rps[hc][:, q * 256:(q + 1) * 256],
                                    b18[:, :, t:t + 1],
                                    w28s[t][:, :, (hc * 2 + q) * 256:(hc * 2 + q + 1) * 256],
                                    start=first and q == 0, stop=False,
                                    perf_mode=DR)
                            first = False
                    if not zb2:
                        b2n = pb.tile([T, H], BF, tag="b2n", name="b2n")
                        if hc == 0:
                            nc.sync.dma_start(b2n[:], b2_d)
                            tdc = pb.tile([T, 2], FP, tag="tdc", name="tdc")
                            tdcb = pb.tile([T, 1], BF, tag="tdcb", name="tdcb")
                            nc.tensor.matmul(tdc[:], tdrow[:].bitcast(FP), ones1[:, :2].bitcast(BF),
                                             start=True, stop=True, skip_group_check=True)
                        # td as column via PSUM transpose trick happens above;
                        # r2 = td @ b2 scaled to the 4096x domain
                        nc.scalar.mul(tdcb[:], tdc[:, :1], PSC)
                        nc.tensor.matmul(rps[hc][:], tdcb[:],
                                         b2n[:, hc * 512:(hc + 1) * 512],
                                         start=zb1, stop=True)
                    else:
                        # close the accumulation group opened by the b1 matmuls
                        nc.tensor.matmul(rps[hc][:, 256:512], b18[:, :, T - 1:T],
                                         w28s[T - 1][:, :, (hc * 2 + 1) * 256:(hc * 2 + 2) * 256],
                                         start=False, stop=True, perf_mode=DR)
                    nc.scalar.copy(rrow[:, hc * 512:(hc + 1) * 512], rps[hc][:])

        # ================= pass C1: W_eff build (fp8 DoubleRow) ==============
        with tc.tile_pool(name="pC1_ps", bufs=8, space="PSUM") as weps:
            for hc in range(2):
                pss = [weps.tile([128, 512], FP, tag="weps", name="weps")
                       for mt in range(KT)]
                for t in range(T):
                    for mt in range(KT):
                        nc.tensor.matmul(
                            pss[mt][:],
                            w18s[t][:, :, mt * 128:(mt + 1) * 128],
                            w28s[t][:, :, hc * 512:(hc + 1) * 512],
                            start=(t == 0), stop=(t == T - 1),
                            perf_mode=DR)
                for mt in range(KT):
                    dst = weff8[mt // 2][:, mt % 2, hc * 512:(hc + 1) * 512]
                    if mt % 2 == 0:
                        nc.vector.tensor_scalar(dst, pss[mt][:], rz2b[:], None,
                                                op0=ALU.mult)
                    else:
                        nc.scalar.activation(dst, pss[mt][:], AF.Identity,
                                             bias=zerot[:], scale=rz2b[:])

        # ================= pass C2: apply + residual + LayerNorm =============
        aps_p = ctx.enter_context(tc.tile_pool(name="pC2_ps", bufs=8, space="PSUM"))
        in_pool = ctx.enter_context(tc.tile_pool(name="in3", bufs=6))
        xt_pool = ctx.enter_context(tc.tile_pool(name="xt3", bufs=4))
        xo_pool = ctx.enter_context(tc.tile_pool(name="xo3", bufs=4))
        sq_pool = ctx.enter_context(tc.tile_pool(name="sq3", bufs=4))
        stats = ctx.enter_context(tc.tile_pool(name="stats", bufs=8))

        in_tiles = {}

        def load_inp(st):
            it = in_pool.tile([128, H], BF, tag="inp", name="inp")
            nc.sync.dma_start(it[:], inp_d[st * 128:(st + 1) * 128, :])
            in_tiles[st] = it

        for st in range(5):
            load_inp(st)

        have_r = not (zb1 and zb2)
        for st in range(NST):
            if st + 5 < NST:
                load_inp(st + 5)
            sw = slice(st * 128, (st + 1) * 128)
            it = in_tiles.pop(st)
            pss = []
            for hc in range(2):
                ps = aps_p.tile([128, 512], FP, tag="xps", name="xps")
                # 4096*input via PE against the scaled identity (opens group)
                nc.tensor.matmul(ps[:], id4096[:],
                                 it[:, hc * 512:(hc + 1) * 512],
                                 start=True, stop=False)
                for j in range(4):
                    kt = hc * 4 + j
                    nc.tensor.matmul(
                        ps[:, j * 128:(j + 1) * 128],
                        htp[kt // 2][:, kt % 2, sw], id4096[:],
                        start=False, stop=False)
                for j in range(NP2):
                    nc.tensor.matmul(
                        ps[:],
                        ht8p[j][:, :, sw],
                        weff8[j][:, :, hc * 512:(hc + 1) * 512],
                        start=False, stop=(j == NP2 - 1 and not have_r),
                        perf_mode=DR)
                if have_r:
                    nc.tensor.matmul(ps[:], ones1[:],
                                     rrow[:, hc * 512:(hc + 1) * 512],
                                     start=False, stop=True)
                pss.append(ps)
            # ---- LayerNorm (x sits in psum at 4096x scale) ----
            xt = xt_pool.tile([128, H], BF, tag="x", name="x")
            s0 = stats.tile([128, 1], FP, tag="s0", name="s0")
            s1 = stats.tile([128, 1], FP, tag="s1", name="s1")
            nc.vector.tensor_scalar(xt[:, 0:512], pss[0][:], IPSC, 0.0,
                                    op0=ALU.mult, op1=ALU.add, accum_out=s0[:])
            nc.scalar.activation(xt[:, 512:1024], pss[1][:], AF.Identity,
                                 bias=zerot[:], scale=IPSC, accum_out=s1[:])
            sq = sq_pool.tile([128, H], BF, tag="sq", name="sq")
            sqa = stats.tile([128, 1], FP, tag="sqa", name="sqa")
            sqb = stats.tile([128, 1], FP, tag="sqb", name="sqb")
            nc.scalar.activation(sq[:, 0:512], xt[:, 0:512], AF.Square,
                                 bias=zerot[:], accum_out=sqa[:])
            nc.vector.scalar_tensor_tensor(sq[:, 512:1024], xt[:, 512:1024], 1.0,
                                           xt[:, 512:1024], op0=ALU.mult,
                                           op1=ALU.mult, accum_out=sqb[:])
            # var = (ssq - ssum^2/H)/H; mu = ssum/H
            ssum = stats.tile([128, 1], FP, tag="ssum", name="ssum")
            nc.vector.tensor_add(ssum[:], s0[:], s1[:])
            q_ = stats.tile([128, 1], FP, tag="q", name="q")
            nc.vector.scalar_tensor_tensor(q_[:], ssum[:], 1.0 / H, ssum[:],
                                           op0=ALU.mult, op1=ALU.mult)
            mu = stats.tile([128, 1], FP, tag="mu", name="mu")
            nc.vector.tensor_scalar(mu[:], ssum[:], 1.0 / H, None, op0=ALU.mult)
            v_ = stats.tile([128, 1], FP, tag="v", name="v")
            nc.vector.scalar_tensor_tensor(v_[:], q_[:], -1.0, sqa[:],
                                           op0=ALU.mult, op1=ALU.add)
            v2_ = stats.tile([128, 1], FP, tag="v2", name="v2")
            nc.vector.tensor_add(v2_[:], v_[:], sqb[:])
            sd = stats.tile([128, 1], FP, tag="sd", name="sd")
            nc.scalar.activation(sd[:], v2_[:], AF.Sqrt, bias=epst[:], scale=1.0 / H)
            isd = stats.tile([128, 1], FP, tag="isd", name="isd")
            nc.vector.reciprocal(isd[:], sd[:])
            xo = xo_pool.tile([128, H], BF, tag="xo", name="xo")
            if zg and zb:
                # xnorm alternates engines to balance DVE vs scalar load
                if st % 2 == 0:
                    nc.vector.tensor_scalar(xo[:], xt[:], mu[:], isd[:],
                                            op0=ALU.subtract, op1=ALU.mult)
                else:
                    nmi = stats.tile([128, 1], FP, tag="nmi", name="nmi")
                    nc.vector.scalar_tensor_tensor(nmi[:], mu[:], -1.0, isd[:],
                                                   op0=ALU.mult, op1=ALU.mult)
                    nc.scalar.activation(xo[:], xt[:], AF.Identity,
                                         bias=nmi[:], scale=isd[:])
            else:
                nc.vector.tensor_scalar(xt[:], xt[:], mu[:], isd[:],
                                        op0=ALU.subtract, op1=ALU.mult)
                if not zg:
                    nc.vector.scalar_tensor_tensor(xt[:], xt[:], 1.0, lngb[:],
                                                   op0=ALU.mult, op1=ALU.mult)
                if not zb:
                    nc.gpsimd.tensor_add(xt[:], xt[:], lnbb[:])
                nc.scalar.copy(xo[:], xt[:])
            nc.sync.dma_start(outp[sw, :], xo[:])

    nc.finalize()
    return nc


_CACHE = {}


def _get_nc(S=S_FULL, zmask=True, zb1=True, zb2=True, zg=True, zb=True):
    key = (S, zmask, zb1, zb2, zg, zb)
    if key not in _CACHE:
        _CACHE[key] = build_nc(S, zmask=zmask, zb1=zb1, zb2=zb2, zg=zg, zb=zb)
    return _CACHE[key]


def _flags(inputs):
    f32 = lambda x: np.asarray(x, dtype=np.float32)
    return dict(
        zmask=not np.any(f32(inputs["attention_mask"])),
        zb1=not np.any(f32(inputs["pal_b1"])),
        zb2=not np.any(f32(inputs["pal_b2"])),
        zg=bool(np.all(f32(inputs["ln_g"]) == 1.0)),
        zb=not np.any(f32(inputs["ln_b"])),
    )


def _in_maps(inputs, S=S_FULL):
    f32 = lambda x: np.ascontiguousarray(np.asarray(x), dtype=np.float32)
    flags = _flags(inputs)
    hs = f32(inputs["hidden_states"])
    inp = f32(inputs["input_tensor"])
    msk = f32(inputs["attention_mask"]).reshape(B, S)
    Wd = np.ascontiguousarray(
        f32(inputs["dense_W"]).reshape(KT, 128, H).transpose(1, 0, 2)).astype(bf16)
    dbias = f32(inputs["dense_b"]).reshape(KT, 128).T.copy()
    W1 = f32(inputs["pal_W1"])   # [T, H, P]
    W2 = f32(inputs["pal_W2"])   # [T, P, H]
    w18 = np.ascontiguousarray(
        (W1.transpose(0, 2, 1) * WSCALE).reshape(T, 2, 128, H).transpose(0, 2, 1, 3)
    ).astype(f8e4)
    w28 = np.ascontiguousarray(
        (W2 * WSCALE).reshape(T, 2, 128, H).transpose(0, 2, 1, 3)
    ).astype(f8e4)
    encw = f32(inputs["enc_W"]).reshape(KT, 128).T.astype(bf16)
    selw = np.ascontiguousarray(
        f32(inputs["sel_W"]).reshape(T, KT, 128).transpose(2, 1, 0)).astype(bf16)
    selb = f32(inputs["sel_b"]).reshape(1, T).astype(bf16)
    shared = dict(Wd=Wd, dbias=dbias, w18=w18, w28=w28, encw=encw,
                  selw=selw, selb=selb)
    if not flags["zb1"]:
        shared["b18"] = np.ascontiguousarray(
            (f32(inputs["pal_b1"]) * WSCALE).reshape(T, 2, 128).transpose(2, 1, 0)
        ).astype(f8e4)
    if not flags["zb2"]:
        shared["b2"] = f32(inputs["pal_b2"]).astype(bf16)
    if not flags["zg"]:
        shared["lng"] = f32(inputs["ln_g"]).reshape(1, H)
    if not flags["zb"]:
        shared["lnb"] = f32(inputs["ln_b"]).reshape(1, H)
    maps = []
    for b in range(B):
        m = dict(hsT=np.ascontiguousarray(
                     hs[b].T.reshape(KT, 128, S // 512, 512)
                     .transpose(2, 1, 0, 3)).astype(bf16),
                 inp=inp[b].astype(bf16), **shared)
        if not flags["zmask"]:
            m["mask"] = msk[b:b + 1]
        maps.append(m)
    return maps


def kernel(**inputs):
    nc = _get_nc(**_flags(inputs))
    res = run_bass_kernel_spmd(nc, _in_maps(inputs), list(range(N_CORES)))
    out = np.stack([np.asarray(res.results[b]["out"], dtype=np.float32)
                    for b in range(B)], axis=0)
    return out
